# revision 7
# baseline (speedup 1.0000x reference)
"""GCN (2x GCNConv + LeakyReLU + Linear) on 8 Trainium2 NeuronCores, v2.

Nodes partitioned contiguously across 8 cores (12500 real + 44 pad ->
12544 = 98 tiles of 128). Edges assigned by destination shard. Tables
store dinv-scaled rows (x~ = dinv*x on host; a~ = dinv*a on device) so
aggregation is an unweighted sum; dinv[dst] applied per output row; the
self-loop term is added locally (no gather).

Message pass: edges sorted by (dst tile, src bucket), padded to 128-edge
chunks per (tile,bucket). Per (group of S tiles, bucket): one dma_gather
(exact edge rows, 256B each, rotating over 4 SWDGE queues so descriptor
generation parallelizes across Q7 cores and the ring never stalls), one
is_equal one-hot build (dst-local id vs iota), then one 128x128x64
matmul per chunk accumulating the segment sum per dst tile in PSUM.
Layer 2 exploits linearity: aggregate first, then fused [64,4] GEMM
(W2@Wfc) on the transposed aggregate.
"""
import sys
import os

sys.path.insert(0, "/opt/trn_rl_repo")

import numpy as np

import concourse.bass as bass
import concourse.mybir as mybir
import concourse.tile as tile
import concourse.bacc as bacc
from concourse.bass_utils import run_bass_kernel_spmd
from concourse.library_config import mlp
from concourse.masks import make_identity

P = 128
NC = 8
NEG = 0.01
NBUCK = 4
NQ = 4  # SWDGE queues

F32 = mybir.dt.float32
BF16 = mybir.dt.bfloat16
I16 = mybir.dt.int16


class Cfg:
    def __init__(self, n_nodes, chunks_tb, s_tiles, nsh_tiles):
        self.n_nodes = n_nodes
        self.d_in = 128
        self.d_mid = 64
        self.d_out = 4
        self.s = s_tiles
        self.nsh_t = nsh_tiles
        self.nsh = nsh_tiles * P
        self.nreal = -(-n_nodes // NC)
        assert self.nreal < self.nsh
        self.npad = NC * self.nsh
        self.buck = 2 * self.nsh
        assert self.buck <= 32768
        assert nsh_tiles % s_tiles == 0
        self.ngrp = nsh_tiles // s_tiles
        self.ngrp0 = self.ngrp // 2     # groups in first half-table
        self.nh0 = self.ngrp0 * s_tiles  # tiles in first half
        self.split = self.ngrp0 > 0
        assert (self.nh0 * NC * P <= 2 * self.buck) and                ((nsh_tiles - self.nh0) * NC * P <= 2 * self.buck)
        self.zrow = self.nreal           # in-bucket index of a zero row
        self.chunks_tb = chunks_tb       # [nsh_t, NBUCK] chunks per tile+bucket
        # per-(group,bucket) call size in chunks
        ct = chunks_tb.reshape(self.ngrp, s_tiles, NBUCK)
        self.c_gb = ct.sum(axis=1)                     # [ngrp, NBUCK]
        assert (self.c_gb * P <= 16368).all(), "gather call exceeds ring"
        # chunk offset of tile t within call (g,b)
        self.off_tb = np.zeros((nsh_tiles, NBUCK), np.int64)
        for g in range(self.ngrp):
            for b in range(NBUCK):
                off = 0
                for ti in range(s_tiles):
                    t = g * s_tiles + ti
                    self.off_tb[t, b] = off
                    off += chunks_tb[t, b]
        # global chunk base of call (g,b), call order g-major then b
        self.call_base = np.zeros((self.ngrp, NBUCK), np.int64)
        acc = 0
        for g in range(self.ngrp):
            for b in range(NBUCK):
                self.call_base[g, b] = acc
                acc += self.c_gb[g, b]
        self.ctot = int(acc)            # total chunks

    def key(self):
        return (self.n_nodes, self.nsh_t, self.s, self.chunks_tb.tobytes())


def build_nc(cfg: Cfg):
    c = cfg
    D = c.d_mid
    S = c.s
    nc = bacc.Bacc("TRN2", target_bir_lowering=False, debug=False,
                   num_devices=NC, num_swdge_queues=NQ)
    t_xT = nc.dram_tensor("xT", [c.d_in, c.nsh], BF16, kind="ExternalInput")
    t_w1 = nc.dram_tensor("w1", [c.d_in, D], BF16, kind="ExternalInput")
    t_b1 = nc.dram_tensor("b1rep", [P, D], F32, kind="ExternalInput")
    t_wc = nc.dram_tensor("wcomb", [D, c.d_out], BF16, kind="ExternalInput")
    t_bc = nc.dram_tensor("bcombT", [c.d_out, 1], F32, kind="ExternalInput")
    t_dinv = nc.dram_tensor("dinvc", [P, c.nsh_t], F32, kind="ExternalInput")
    t_iota = nc.dram_tensor("iota", [P, P], BF16, kind="ExternalInput")
    t_idx = nc.dram_tensor("idx16", [P, c.ctot * 8], I16, kind="ExternalInput")
    t_did = nc.dram_tensor("dstid", [P, c.ctot], BF16, kind="ExternalInput")
    t_out = nc.dram_tensor("out", [c.d_out, c.nsh], F32, kind="ExternalOutput")

    with tile.TileContext(nc) as tc:
        with (
            tc.tile_pool(name="const", bufs=1) as cp,
            tc.tile_pool(name="sb", bufs=2) as sbp,
            tc.tile_pool(name="gx", bufs=1) as gxp,
            tc.tile_pool(name="gxb", bufs=1) as gxbp,
            tc.tile_pool(name="oh", bufs=1) as ohp,
            tc.tile_pool(name="ps_h", bufs=2, space="PSUM") as ps_h,
            tc.tile_pool(name="ps_agg", bufs=1, space="PSUM") as ps_agg,
            tc.tile_pool(name="ps_t", bufs=1, space="PSUM") as ps_t,
            tc.tile_pool(name="ps_o", bufs=1, space="PSUM") as ps_o,
            tc.tile_pool(name="dram", bufs=1, space="DRAM") as dp,
        ):
            nc.gpsimd.load_library(mlp)

            ident = cp.tile([P, P], F32)
            make_identity(nc, ident[:])
            w1_sb = cp.tile([c.d_in, D], BF16)
            nc.sync.dma_start(w1_sb[:], t_w1[:])
            b1_sb = cp.tile([P, D], F32)
            nc.sync.dma_start(b1_sb[:], t_b1[:])
            wc_sb = cp.tile([D, c.d_out], BF16)
            nc.sync.dma_start(wc_sb[:], t_wc[:])
            bcT_sb = cp.tile([c.d_out, 1], F32)
            nc.sync.dma_start(bcT_sb[:], t_bc[:])
            dinv_sb = cp.tile([P, c.nsh_t], F32)
            nc.sync.dma_start(dinv_sb[:], t_dinv[:])
            iota_sb = cp.tile([P, P], BF16)
            nc.sync.dma_start(iota_sb[:], t_iota[:])

            # persistent local tables (tile-major)
            hloc = cp.tile([P, c.nsh_t, D], F32)   # h~ = dinv * (x W1)
            aloc = cp.tile([P, c.nsh_t, D], F32)   # a~ = dinv * LeakyReLU(..)

            nh0 = c.nh0 if c.split else c.nsh_t
            nh1 = c.nsh_t - nh0
            h_sh = [dp.tile([nh0 * P, D], F32, name="h_sh0")]
            a_sh = [dp.tile([nh0 * P, D], F32, name="a_sh0")]
            h_fl = [dp.tile([NC * nh0 * P, D], F32, addr_space="Shared",
                            name="h_fl0")]
            a_fl = [dp.tile([NC * nh0 * P, D], F32, addr_space="Shared",
                            name="a_fl0")]
            if nh1 > 0:
                h_sh.append(dp.tile([nh1 * P, D], F32, name="h_sh1"))
                a_sh.append(dp.tile([nh1 * P, D], F32, name="a_sh1"))
                h_fl.append(dp.tile([NC * nh1 * P, D], F32,
                                    addr_space="Shared", name="h_fl1"))
                a_fl.append(dp.tile([NC * nh1 * P, D], F32,
                                    addr_space="Shared", name="a_fl1"))

            # ---- phase A: hloc = (dinv*x) @ W1 (chunked xT load) ----
            with tc.tile_pool(name="pa", bufs=2) as pa:
                for g in range(c.ngrp):
                    xT_sb = pa.tile([c.d_in, S * P], BF16, tag="xT",
                                    name="xT_sb")
                    nc.sync.dma_start(
                        xT_sb[:], t_xT[:, g * S * P:(g + 1) * S * P])
                    for ti in range(S):
                        t = g * S + ti
                        ph = ps_h.tile([P, D], F32, space="PSUM", tag="ph")
                        nc.tensor.matmul(out=ph[:],
                                         lhsT=xT_sb[:, ti * P:(ti + 1) * P],
                                         rhs=w1_sb[:], start=True, stop=True)
                        nc.vector.tensor_copy(hloc[:, t, :], ph[:])
                nc.sync.dma_start(
                    h_sh[0][:].rearrange("(t p) f -> p t f", p=P),
                    hloc[:, :nh0, :])
                if nh1 > 0:
                    nc.sync.dma_start(
                        h_sh[1][:].rearrange("(t p) f -> p t f", p=P),
                        hloc[:, nh0:, :])

            def msg_pass(tables, loc_tbl, layer, out_sh=None):
                pend = []

                def do_body(g):
                    cg = int(c.c_gb[g].sum())        # chunks this group
                    base = int(c.call_base[g, 0])    # global chunk base
                    ib = sbp.tile([P, cg * 8], I16, tag="ib")
                    nc.sync.dma_start(
                        ib[:], t_idx[:, base * 8:(base + cg) * 8])
                    dd = sbp.tile([P, cg], BF16, tag="dd")
                    nc.sync.dma_start(dd[:], t_did[:, base:base + cg])

                    stage = sbp.tile([P, S, D], F32, tag="stage")
                    # first nonempty bucket per tile (for self-term fold)
                    first_b = {}
                    for ti in range(S):
                        t = g * S + ti
                        for b in range(NBUCK - 1, -1, -1):
                            if c.chunks_tb[t, b] > 0:
                                first_b[ti] = b

                    for b in range(NBUCK):
                        cgb = int(c.c_gb[g, b])
                        if cgb == 0:
                            continue
                        cb = int(c.call_base[g, b]) - base  # chunk off in grp
                        tbl = tables[b // 2] if c.split else tables[0]
                        bb = (b % 2) if c.split else b
                        gx = gxp.tile([P, cgb, D], F32, tag=f"gx{g % 2}{b}")
                        nc.gpsimd.dma_gather(
                            gx[:], tbl[bb * c.buck:(bb + 1) * c.buck, :],
                            ib[:, cb * 8:(cb + cgb) * 8],
                            cgb * P, cgb * P, D, single_packet=False,
                            queue_num=(g * NBUCK + b) % NQ,
                        )
                        gxb = gxbp.tile([P, cgb, D], BF16,
                                        tag=f"gxb{b}")
                        nc.scalar.activation(
                            gxb[:].rearrange("p c f -> p (c f)"),
                            gx[:].rearrange("p c f -> p (c f)"),
                            mybir.ActivationFunctionType.Copy, scale=1.0)
                        oh = ohp.tile([P, cgb, P], BF16, tag=f"oh{b % 2}")
                        nc.vector.tensor_tensor(
                            out=oh[:],
                            in0=dd[:, cb:cb + cgb].unsqueeze(2)
                                .broadcast_to([P, cgb, P]),
                            in1=iota_sb[:].unsqueeze(1)
                                .broadcast_to([P, cgb, P]),
                            op=mybir.AluOpType.is_equal)
                        for ti in range(S):
                            t = g * S + ti
                            nch = int(c.chunks_tb[t, b])
                            if nch == 0:
                                continue
                            off = int(c.off_tb[t, b])
                            aggb = ps_agg.tile([P, D], F32, space="PSUM",
                                               tag=f"agg{ti % 2}",
                                               name=f"agg{ti % 2}")
                            for j in range(nch):
                                nc.tensor.matmul(
                                    out=aggb[:],
                                    lhsT=oh[:, off + j, :],
                                    rhs=gxb[:, off + j, :],
                                    start=(j == 0),
                                    stop=(j == nch - 1))
                            nc.vector.tensor_tensor(
                                out=stage[:, ti, :],
                                in0=(loc_tbl[:, t, :] if first_b[ti] == b
                                     else stage[:, ti, :]),
                                in1=aggb[:],
                                op=mybir.AluOpType.add)
                    pend.append((g, stage))
                    return

                def do_epilogue(g, stage):
                    dv = dinv_sb[:, g * S:g * S + S].unsqueeze(2) \
                        .broadcast_to([P, S, D])
                    nc.vector.tensor_tensor(out=stage[:], in0=stage[:],
                                            in1=dv, op=mybir.AluOpType.mult)
                    if layer == 1:
                        b13 = b1_sb[:].unsqueeze(1).broadcast_to([P, S, D])
                        nc.vector.tensor_tensor(out=stage[:], in0=stage[:],
                                                in1=b13,
                                                op=mybir.AluOpType.add)
                        asc = sbp.tile([P, S, D], F32, tag="asc")
                        nc.scalar.activation(
                            asc[:].rearrange("p s d -> p (s d)"),
                            stage[:].rearrange("p s d -> p (s d)"),
                            mybir.ActivationFunctionType.Copy, scale=NEG)
                        nc.vector.tensor_tensor(out=stage[:], in0=stage[:],
                                                in1=asc[:],
                                                op=mybir.AluOpType.max)
                        nc.vector.tensor_tensor(
                            out=aloc[:, g * S:g * S + S, :], in0=stage[:],
                            in1=dv, op=mybir.AluOpType.mult)
                        if c.split and g >= c.ngrp0:
                            r0 = (g - c.ngrp0) * S * P
                        else:
                            r0 = g * S * P
                        sh = out_sh[1] if (c.split and g >= c.ngrp0) \
                            else out_sh[0]
                        nc.sync.dma_start(
                            sh[r0:r0 + S * P, :]
                                .rearrange("(t p) f -> p t f", p=P),
                            aloc[:, g * S:g * S + S, :])
                        if c.split and g == c.ngrp0 - 1 and layer == 1:
                            nc.gpsimd.collective_compute(
                                "AllGather", mybir.AluOpType.bypass,
                                replica_groups=[list(range(NC))],
                                ins=[out_sh[0].opt()], outs=[a_fl[0].opt()],
                            )
                    else:
                        zg = sbp.tile([D, S * P], BF16, tag="zg")
                        for ti in range(S):
                            pt = ps_t.tile([D, P], F32, space="PSUM",
                                           tag="pt")
                            nc.tensor.transpose(
                                out=pt[:], in_=stage[:, ti, :],
                                identity=ident[:])
                            nc.vector.tensor_copy(
                                zg[:, ti * P:(ti + 1) * P], pt[:])
                        # fused FC on this group's columns (<=512/psum bank)
                        gn0 = g * S * P
                        gn1 = (g + 1) * S * P
                        ob = sbp.tile([c.d_out, S * P], F32, tag="ob")
                        for n0 in range(0, S * P, 512):
                            n1 = min(S * P, n0 + 512)
                            pf = ps_o.tile([c.d_out, 512], F32, space="PSUM",
                                           tag="pf")
                            nc.tensor.matmul(out=pf[:, :n1 - n0],
                                             lhsT=wc_sb[:],
                                             rhs=zg[:, n0:n1],
                                             start=True, stop=True)
                            nc.vector.tensor_tensor(
                                out=ob[:, n0:n1],
                                in0=pf[:, :n1 - n0],
                                in1=bcT_sb[:].broadcast_to(
                                    [c.d_out, n1 - n0]),
                                op=mybir.AluOpType.add)
                        nc.sync.dma_start(t_out[:, gn0:gn1], ob[:])

                for g in range(c.ngrp):
                    do_body(g)
                    if pend and pend[0][0] <= g - 1:
                        do_epilogue(*pend.pop(0))
                while pend:
                    do_epilogue(*pend.pop(0))

            for i in range(len(h_sh)):
                nc.gpsimd.collective_compute(
                    "AllGather", mybir.AluOpType.bypass,
                    replica_groups=[list(range(NC))],
                    ins=[h_sh[i].opt()], outs=[h_fl[i].opt()],
                )
            msg_pass(h_fl, hloc, 1, out_sh=a_sh)

            if c.split:
                # first-half AllGather was emitted inside layer 1
                nc.gpsimd.collective_compute(
                    "AllGather", mybir.AluOpType.bypass,
                    replica_groups=[list(range(NC))],
                    ins=[a_sh[1].opt()], outs=[a_fl[1].opt()],
                )
            else:
                nc.gpsimd.collective_compute(
                    "AllGather", mybir.AluOpType.bypass,
                    replica_groups=[list(range(NC))],
                    ins=[a_sh[0].opt()], outs=[a_fl[0].opt()],
                )
            msg_pass(a_fl, aloc, 2)

    nc.compile()
    return nc


def _bucket_of(src, nreal, nsh_t, s):
    # returns (bucket, in-bucket index) under the half-table layout
    P_ = P
    nsh = nsh_t * P_
    ngrp = nsh_t // s
    ngrp0 = ngrp // 2
    nh0 = ngrp0 * s
    buck = 2 * nsh
    score = src // nreal
    sloc = src % nreal
    stl = sloc // P_
    if ngrp0 > 0:
        h = (stl >= nh0).astype(np.int64)
        rows0 = nh0 * P_
        rows1 = (nsh_t - nh0) * P_
        rin = np.where(h == 0, score * rows0 + sloc,
                       score * rows1 + (sloc - rows0))
        b = h * 2 + rin // buck
        inb = rin % buck
    else:
        gsrc = score * nsh + sloc
        b = gsrc // buck
        inb = gsrc % buck
    return b, inb


def make_cfg(x, edge_index):
    n = x.shape[0]
    nreal = -(-n // NC)
    nsh_t = nreal // P + 1
    s = max(d for d in range(1, nsh_t + 1) if nsh_t % d == 0 and d <= 7)
    nsh = nsh_t * P

    src = np.asarray(edge_index[0], dtype=np.int64)
    dst = np.asarray(edge_index[1], dtype=np.int64)
    core = dst // nreal
    loc = dst - core * nreal
    tl = loc // P
    b = _bucket_of(src, nreal, nsh_t, s)[0]
    cell = (core * nsh_t + tl) * NBUCK + b
    cnt = np.bincount(cell, minlength=NC * nsh_t * NBUCK)
    cnt = cnt.reshape(NC, nsh_t, NBUCK)
    chunks_tb = -(-cnt.max(axis=0) // P)          # [nsh_t, NBUCK]
    empty = chunks_tb.sum(axis=1) == 0
    chunks_tb[empty, 0] = 1
    return Cfg(n, chunks_tb.astype(np.int64), s, nsh_t)


def host_prep(x, edge_index, W1, b1, W2, b2, Wfc, bfc, cfg: Cfg):
    c = cfg
    n = c.n_nodes
    nreal = c.nreal
    src = np.asarray(edge_index[0], dtype=np.int64)
    dst = np.asarray(edge_index[1], dtype=np.int64)

    import ml_dtypes
    deg = np.bincount(dst, minlength=n).astype(np.float64) + 1.0
    dinv = (1.0 / np.sqrt(deg)).astype(np.float32)

    core = dst // nreal
    loc = dst - core * nreal
    tl = loc // P
    d_loc = loc % P
    b, inb64 = _bucket_of(src, nreal, c.nsh_t, c.s)
    inb = inb64.astype(np.int16)

    # slot of each edge in the padded chunk stream (shared layout)
    cell = (core * c.nsh_t + tl) * NBUCK + b
    order = np.lexsort((inb, cell))
    cell_s = cell[order]
    nbins = NC * c.nsh_t * NBUCK
    start = np.searchsorted(cell_s, np.arange(nbins))
    rank = np.arange(len(cell_s)) - start[cell_s]
    rank_e = np.empty(len(cell), np.int64)
    rank_e[order] = rank

    # chunk base per (t,b): call_base[g,b] + off_tb[t,b]
    g_of_t = np.arange(c.nsh_t) // c.s
    cb_tb = c.call_base[g_of_t, :] + c.off_tb        # [nsh_t, NBUCK] chunks
    slot = cb_tb[tl, b] * P + rank_e

    nslots = c.ctot * P
    idx_arr = np.full((NC, nslots), 0, np.int16)
    did_arr = np.full((NC, nslots), 999.0, np.float32)  # cast to bf16 below
    idx_arr[core, slot] = inb
    did_arr[core, slot] = d_loc.astype(np.float32)

    # idx16: [NC, 16, nslots/16] wrap, tiled to 128 partitions
    idx16 = np.ascontiguousarray(
        idx_arr.reshape(NC, c.ctot * 8, 16).transpose(0, 2, 1))
    idx16 = np.tile(idx16, (1, 8, 1))
    # dstid: [NC, 128, ctot]
    did_t = np.ascontiguousarray(
        did_arr.reshape(NC, c.ctot, P).transpose(0, 2, 1)
        .astype(ml_dtypes.bfloat16))

    x = np.asarray(x, dtype=np.float32) * dinv[:, None]
    xT = np.zeros((NC, c.d_in, c.nsh), np.float32)  # cast to bf16 per core
    dinvc = np.zeros((NC, P, c.nsh_t), np.float32)
    for ci in range(NC):
        r0 = ci * nreal
        r1 = min(n, r0 + nreal)
        xT[ci, :, :r1 - r0] = x[r0:r1].T
        dv = np.zeros(c.nsh, np.float32)
        dv[:r1 - r0] = dinv[r0:r1]
        dinvc[ci] = dv.reshape(c.nsh_t, P).T

    W1 = np.ascontiguousarray(
        np.asarray(W1, np.float32).astype(ml_dtypes.bfloat16))
    b1rep = np.tile(np.asarray(b1, np.float32)[None, :], (P, 1))
    wcomb = np.ascontiguousarray(
        (np.asarray(W2, np.float32) @ np.asarray(Wfc, np.float32))
        .astype(ml_dtypes.bfloat16))
    bcomb = (np.asarray(b2, np.float32) @ np.asarray(Wfc, np.float32)
             + np.asarray(bfc, np.float32))
    bcombT = np.ascontiguousarray(bcomb[:, None])
    iota_m = np.ascontiguousarray(
        np.tile(np.arange(P, dtype=np.float32)[None, :],
                (P, 1)).astype(ml_dtypes.bfloat16))

    in_maps = []
    for ci in range(NC):
        in_maps.append({
            "xT": np.ascontiguousarray(xT[ci].astype(ml_dtypes.bfloat16)),
            "w1": W1, "b1rep": b1rep, "wcomb": wcomb, "bcombT": bcombT,
            "dinvc": np.ascontiguousarray(dinvc[ci]),
            "iota": iota_m,
            "idx16": np.ascontiguousarray(idx16[ci]),
            "dstid": np.ascontiguousarray(did_t[ci]),
        })
    return in_maps


_NC_CACHE = {}


def kernel(x, edge_index, W1, b1, W2, b2, Wfc, bfc):
    x = np.asarray(x)
    edge_index = np.asarray(edge_index)
    n = x.shape[0]

    cfg = make_cfg(x, edge_index)
    key = cfg.key()
    if key not in _NC_CACHE:
        _NC_CACHE[key] = build_nc(cfg)
    nc = _NC_CACHE[key]

    in_maps = host_prep(x, edge_index, W1, b1, W2, b2, Wfc, bfc, cfg)
    res = run_bass_kernel_spmd(nc, in_maps, core_ids=list(range(NC)))

    outs = []
    left = n
    for ci in range(NC):
        take = min(cfg.nreal, left)
        outs.append(res.results[ci]["out"].T[:take])
        left -= take
    return np.ascontiguousarray(
        np.concatenate(outs, axis=0)).astype(np.float32)
